# revision 17
# baseline (speedup 1.0000x reference)
"""VQ codebook nearest-neighbor kernel for Trainium2 (8 NeuronCores, SPMD).

Problem: z [16, 64, 128, 128] f32, emb [256, 64] f32 ->
         codes [16, 128, 128] int32 = argmin_k ||x_p - emb_k||_2
         (x = z rearranged 't a b c -> t (b c) a').

Strategy (data-parallel over t, 2 t-slices per core):
  - argmin_k dist = argmin_k (|e_k|^2 - 2 x.e_k)  [x^2 and sqrt are
    monotone per-point -> dropped].
  - Inputs are split host-side into bf16 hi+mid pairs (Dekker-style):
    z = zh + zm + O(2^-17), -2*emb^T = eh + em + O(2^-17). Three bf16
    matmul passes per 128-point tile (zh.eh, zh.em, zm.eh) reproduce the
    fp32 product to ~1e-4 at 1 cycle/column on the PE (vs 4 for fp32).
  - The per-k constant |e_k|^2 + B_BIG is split into three bf16 parts
    a+b+c riding each pass's ones-row. PSUM accumulation in fp32 at
    magnitude ~3072 quantizes w+B_BIG to 2^-12 (B_BIG pins the binade).
  - Two bf16 rank-1 accumulation passes: -B_BIG (recenter, exact) then
    +(k%16)*2^-16 (pack the 4-bit within-chunk index below the quantum;
    exact because |w_q| < 256 -> 24 mantissa bits suffice).
  - One DVE reduce_min over 16-wide k-chunks -> 16 packed chunk minima
    per point. Level-2 on [*, 16]: global min, equality bitmask dotted
    with 2^j, lowest-set-bit -> chunk index j*; payload of the min ->
    i*; code = 16*j* + i*.
Ties resolve to the smallest k (payload ordering + lowest-set-bit),
matching jnp.argmin's first-index semantics.
"""

import sys

for _p in ("/opt/trn_rl_repo", "/root/.axon_site/_ro/trn_rl_repo"):
    if _p not in sys.path:
        sys.path.insert(0, _p)

import numpy as np

import concourse.bass as bass
import concourse.bacc as bacc
import concourse.mybir as mybir
from concourse import tile
from concourse.bass_utils import run_bass_kernel_spmd

F32 = mybir.dt.float32
BF16 = mybir.dt.bfloat16
I32 = mybir.dt.int32

N_CORES = 8
T_TOTAL = 16
N_SLICES = T_TOTAL // N_CORES          # t-slices per core
POINTS = 128 * 128                     # points per t-slice
N_CHUNKS = 4                           # point-chunks per slice
CHUNK_PTS = POINTS // N_CHUNKS         # 4096
TILES_PER_CHUNK = CHUNK_PTS // 128     # 32
K = 256
D = 64

B_BIG = 3072.0                         # 2^11+2^10: w+B in [2048,4096) -> quantum 2^-12
DELTA = 2.0 ** -16                     # payload step for i = k % 16
PAYSCALE = 2.0 ** 16

AluOp = mybir.AluOpType
Axis = mybir.AxisListType


def _build_nc():
    nc = bacc.Bacc(None, target_bir_lowering=False, debug=False)

    zp_d = nc.declare_dram_parameter(
        "z_planes", [N_SLICES, 2, D + 2, POINTS], BF16, isOutput=False
    )
    mov_d = nc.declare_dram_parameter("mov", [3, D + 2, K], BF16, isOutput=False)
    c23_d = nc.declare_dram_parameter("c23", [1, 2 * K], BF16, isOutput=False)
    pow2_d = nc.declare_dram_parameter("pow2", [128, 16], F32, isOutput=False)
    pay_d = nc.declare_dram_parameter("paycst", [128, 2 * K], F32, isOutput=False)
    codes_d = nc.declare_dram_parameter(
        "codes", [N_SLICES, 128, N_CHUNKS, TILES_PER_CHUNK], I32, isOutput=True
    )

    with tile.TileContext(nc) as tc:
        with (
            tc.tile_pool(name="cst", bufs=1) as cst_pool,
            tc.tile_pool(name="padd", bufs=6) as padd_pool,
            tc.tile_pool(name="chunk", bufs=3) as chunk_pool,
            tc.tile_pool(name="psum", bufs=8, space="PSUM") as psum_pool,
            tc.tile_pool(name="m16", bufs=2) as m16_pool,
            tc.tile_pool(name="l2", bufs=2) as l2_pool,
            tc.tile_pool(name="codes", bufs=2) as codes_pool,
        ):
            m1 = cst_pool.tile([D + 2, K], BF16, tag="m1")
            m2 = cst_pool.tile([D + 2, K], BF16, tag="m2")
            m3 = cst_pool.tile([D + 2, K], BF16, tag="m3")
            # rank-1 moving row lives at partition D (=64) to match the
            # ones-row stationary's base partition (matmul requirement).
            cr = cst_pool.tile([D + 1, 2 * K], BF16, tag="cr")
            c3 = cr[D : D + 1, 0 : 2 * K]
            pow2 = cst_pool.tile([128, 16], F32, tag="pow2")
            paycst = cst_pool.tile([128, 2 * K], F32, tag="paycst")
            nc.sync.dma_start(m1[:], mov_d[0])
            nc.sync.dma_start(m2[:], mov_d[1])
            nc.sync.dma_start(m3[:], mov_d[2])
            nc.sync.dma_start(c3, c23_d[:])
            nc.sync.dma_start(pow2[:], pow2_d[:])
            nc.sync.dma_start(paycst[:], pay_d[:])

            for s in range(N_SLICES):
                codes_sb = codes_pool.tile([128, N_CHUNKS * TILES_PER_CHUNK], I32)
                for c in range(N_CHUNKS):
                    ch_hi = chunk_pool.tile([D + 2, CHUNK_PTS], BF16, tag="ch_hi")
                    ch_mid = chunk_pool.tile([D + 2, CHUNK_PTS], BF16, tag="ch_mid")
                    rng = slice(c * CHUNK_PTS, (c + 1) * CHUNK_PTS)
                    nc.sync.dma_start(ch_hi[:], zp_d[s, 0, :, rng])
                    nc.sync.dma_start(ch_mid[:], zp_d[s, 1, :, rng])
                    # [65, 32, 128]: [:, j, m] = column j + 32*m
                    hi_v = ch_hi[:].rearrange("p (n j) -> p j n", j=TILES_PER_CHUNK)
                    mid_v = ch_mid[:].rearrange("p (n j) -> p j n", j=TILES_PER_CHUNK)
                    ones_ap = hi_v[D : D + 1, 0, :]

                    m16 = m16_pool.tile([128, TILES_PER_CHUNK, 16], F32)
                    for p in range(TILES_PER_CHUNK // 2):
                        ps = psum_pool.tile([128, 512], F32)
                        regs = [ps[:, 0:256], ps[:, 256:512]]
                        js = [2 * p, 2 * p + 1]
                        # Interleave the two tiles' passes so consecutive
                        # matmuls hit different PSUM cells (drain hides
                        # under the other tile's fill). Groups overlap in
                        # the bank -> skip the sim's group lint.
                        for h in range(2):
                            # start=True zeroes the WHOLE bank, so only the
                            # first matmul of the bank may carry it; the
                            # second tile's first write lands on zeroed
                            # cells (has_written=0 -> store).
                            nc.tensor.matmul(
                                regs[h],
                                hi_v[0 : D + 1, js[h], :],
                                m1[0 : D + 1, :],
                                start=(h == 0),
                                stop=False,
                                skip_group_check=True,
                            )
                        for h in range(2):
                            nc.tensor.matmul(
                                regs[h],
                                hi_v[0 : D + 1, js[h], :],
                                m2[0 : D + 1, :],
                                start=False,
                                stop=False,
                                skip_group_check=True,
                            )
                        # 66-row pass: rows 64/65 carry c_k and -B_BIG.
                        # The -B_BIG row accumulates last (ascending row
                        # order), quantizing this pass's content at 2^-12
                        # before the PSUM recenter.
                        route = "VVGGGGGG"[p % 8]
                        for h in range(2):
                            nc.tensor.matmul(
                                regs[h],
                                mid_v[:, js[h], :],
                                m3[:],
                                start=False,
                                stop=(h == 1),
                                skip_group_check=True,
                            )
                        mslc = m16[:, 2 * p : 2 * p + 2, :]
                        if route == "V":
                            # DVE carries the payload add.
                            sc = padd_pool.tile([128, 512], F32, tag="scv")
                            nc.vector.tensor_tensor(
                                sc[:], ps[:], paycst[:], op=AluOp.add
                            )
                        else:
                            # ACT copies PSUM out, GpSimd adds the payload;
                            # DVE only reduces (its PSUM port stays free).
                            sc1 = padd_pool.tile([128, 512], F32, tag="sc1")
                            nc.scalar.activation(
                                sc1[:], ps[:], mybir.ActivationFunctionType.Copy
                            )
                            sc = padd_pool.tile([128, 512], F32, tag="sc2")
                            nc.gpsimd.tensor_tensor(
                                sc[:], sc1[:], paycst[:], op=AluOp.add
                            )
                        nc.vector.tensor_reduce(
                            mslc,
                            sc[:].rearrange("m (t c i) -> m t c i", t=2, c=16, i=16),
                            axis=Axis.X,
                            op=AluOp.min,
                        )

                    # ---- level 2: 32 tiles at once ----
                    vmin = l2_pool.tile([128, TILES_PER_CHUNK], F32)
                    nc.vector.tensor_reduce(vmin[:], m16[:], axis=Axis.X, op=AluOp.min)
                    eq = l2_pool.tile([128, TILES_PER_CHUNK, 16], F32)
                    nc.vector.tensor_tensor(
                        eq[:],
                        m16[:],
                        vmin[:].unsqueeze(2).broadcast_to([128, TILES_PER_CHUNK, 16]),
                        op=AluOp.is_equal,
                    )
                    bs = l2_pool.tile([128, TILES_PER_CHUNK, 16], F32)
                    nc.vector.tensor_tensor(
                        bs[:],
                        eq[:],
                        pow2[:].unsqueeze(1).broadcast_to([128, TILES_PER_CHUNK, 16]),
                        op=AluOp.mult,
                    )
                    b = l2_pool.tile([128, TILES_PER_CHUNK], F32)
                    nc.vector.tensor_reduce(b[:], bs[:], axis=Axis.X, op=AluOp.add)
                    bi = l2_pool.tile([128, TILES_PER_CHUNK], I32)
                    nc.vector.tensor_copy(bi[:], b[:])
                    nbi = l2_pool.tile([128, TILES_PER_CHUNK], I32)
                    nc.vector.tensor_scalar(nbi[:], b[:], -1.0, None, AluOp.mult)
                    low = l2_pool.tile([128, TILES_PER_CHUNK], I32)
                    nc.vector.tensor_tensor(low[:], bi[:], nbi[:], op=AluOp.bitwise_and)
                    lowf = l2_pool.tile([128, TILES_PER_CHUNK], F32)
                    nc.vector.tensor_copy(lowf[:], low[:])
                    # j* = (float_bits(2^j) >> 23) - 127 ; jv = 16*j*
                    jt = l2_pool.tile([128, TILES_PER_CHUNK], I32)
                    nc.vector.tensor_scalar(
                        jt[:], lowf[:].bitcast(I32), 23, None, AluOp.arith_shift_right
                    )
                    jv = l2_pool.tile([128, TILES_PER_CHUNK], I32)
                    nc.vector.tensor_scalar(
                        jv[:], jt[:], 127, 16, AluOp.subtract, AluOp.mult
                    )
                    # i* = (int(vmin * 2^16)) & 15
                    t1 = l2_pool.tile([128, TILES_PER_CHUNK], I32)
                    nc.vector.tensor_scalar(t1[:], vmin[:], PAYSCALE, None, AluOp.mult)
                    t2 = l2_pool.tile([128, TILES_PER_CHUNK], I32)
                    nc.vector.tensor_scalar(t2[:], t1[:], 15, None, AluOp.bitwise_and)
                    nc.vector.tensor_tensor(
                        codes_sb[:, c * TILES_PER_CHUNK : (c + 1) * TILES_PER_CHUNK],
                        jv[:],
                        t2[:],
                        op=AluOp.add,
                    )
                nc.sync.dma_start(
                    codes_d[s],
                    codes_sb[:].rearrange("m (c j) -> m c j", c=N_CHUNKS),
                )
    nc.compile()
    return nc


def _bf16_rn(v: np.ndarray) -> np.ndarray:
    """Round fp32 to bf16 (round-to-nearest-even), returned as fp32 values."""
    u = np.ascontiguousarray(v, dtype=np.float32).view(np.uint32)
    r = (u + np.uint32(0x7FFF) + ((u >> np.uint32(16)) & np.uint32(1))) & np.uint32(
        0xFFFF0000
    )
    return r.view(np.float32)


def _to_bf16(v: np.ndarray) -> np.ndarray:
    import ml_dtypes

    return np.asarray(v, dtype=np.float32).astype(ml_dtypes.bfloat16)


def _make_consts(emb: np.ndarray):
    e2 = (emb.astype(np.float64) ** 2).sum(axis=-1)
    E = (-2.0 * emb.T).astype(np.float32)          # [64, 256]
    eh = _bf16_rn(E)
    em = _bf16_rn(E - eh)
    v64 = (e2 + B_BIG).astype(np.float64)
    av = _bf16_rn(v64.astype(np.float32))
    bv = _bf16_rn((v64 - av.astype(np.float64)).astype(np.float32))
    cv = _bf16_rn((v64 - av.astype(np.float64) - bv).astype(np.float32))

    mov = np.zeros((3, D + 2, K), dtype=np.float32)
    mov[0, 0:D] = eh
    mov[0, D] = av
    mov[1, 0:D] = em
    mov[1, D] = bv
    mov[2, 0:D] = eh
    mov[2, D] = cv
    mov[2, D + 1] = -B_BIG

    c23 = np.zeros((1, 2 * K), dtype=np.float32)
    c23[0, :] = (np.arange(2 * K) % 16).astype(np.float32) * DELTA

    pow2 = np.broadcast_to(
        (2.0 ** np.arange(16)).astype(np.float32), (128, 16)
    ).copy()
    paycst = np.broadcast_to(c23[0:1, :], (128, 2 * K)).copy()
    return _to_bf16(mov), _to_bf16(c23), pow2, paycst


def _pack_z(zr: np.ndarray) -> np.ndarray:
    """zr [S, 64, POINTS] f32 -> [S, 2, 65, POINTS] bf16 (hi/mid planes,
    ones row at partition 64 of both planes)."""
    S = zr.shape[0]
    zh = _bf16_rn(zr)
    zm = _bf16_rn(zr - zh)
    out = np.ones((S, 2, D + 2, POINTS), dtype=np.float32)
    out[:, 0, 0:D] = zh
    out[:, 1, 0:D] = zm
    return _to_bf16(out)


def _run(z: np.ndarray, emb: np.ndarray, **spmd_kwargs):
    z = np.asarray(z, dtype=np.float32)
    emb = np.asarray(emb, dtype=np.float32)
    t, a, b, c = z.shape
    assert (t, a, b, c) == (16, 64, 128, 128) and emb.shape == (256, 64)

    zr = z.reshape(t, a, b * c)
    mov, c23, pow2, paycst = _make_consts(emb)

    nc = _build_nc()
    in_maps = []
    for i in range(N_CORES):
        shard = _pack_z(zr[i * N_SLICES : (i + 1) * N_SLICES])
        in_maps.append(
            {
                "z_planes": shard,
                "mov": mov,
                "c23": c23,
                "pow2": pow2,
                "paycst": paycst,
            }
        )
    res = run_bass_kernel_spmd(nc, in_maps, core_ids=list(range(N_CORES)), **spmd_kwargs)

    out = np.empty((t, b * c), dtype=np.int32)
    for i in range(N_CORES):
        arr = np.asarray(res.results[i]["codes"])  # [N_SLICES, 128, N_CHUNKS, 32]
        # point p = 4096*c + 32*m + j  ->  [s, c, m, j] order is p-major
        out[i * N_SLICES : (i + 1) * N_SLICES] = (
            arr.transpose(0, 2, 1, 3).reshape(N_SLICES, b * c).astype(np.int32)
        )
    return out.reshape(t, b, c), res


def kernel(z: np.ndarray, emb: np.ndarray) -> np.ndarray:
    return _run(z, emb)[0]


# revision 18
# speedup vs baseline: 1.0054x; 1.0054x over previous
"""VQ codebook nearest-neighbor kernel for Trainium2 (8 NeuronCores, SPMD).

Problem: z [16, 64, 128, 128] f32, emb [256, 64] f32 ->
         codes [16, 128, 128] int32 = argmin_k ||x_p - emb_k||_2
         (x = z rearranged 't a b c -> t (b c) a').

Strategy (data-parallel over t, 2 t-slices per core):
  - argmin_k dist = argmin_k (|e_k|^2 - 2 x.e_k)  [x^2 and sqrt are
    monotone per-point -> dropped].
  - Inputs are split host-side into bf16 hi+mid pairs (Dekker-style):
    z = zh + zm + O(2^-17), -2*emb^T = eh + em + O(2^-17). Three bf16
    matmul passes per 128-point tile (zh.eh, zh.em, zm.eh) reproduce the
    fp32 product to ~1e-4 at 1 cycle/column on the PE (vs 4 for fp32).
  - The per-k constant |e_k|^2 + B_BIG is split into three bf16 parts
    a+b+c riding each pass's ones-row. PSUM accumulation in fp32 at
    magnitude ~3072 quantizes w+B_BIG to 2^-12 (B_BIG pins the binade).
  - Two bf16 rank-1 accumulation passes: -B_BIG (recenter, exact) then
    +(k%16)*2^-16 (pack the 4-bit within-chunk index below the quantum;
    exact because |w_q| < 256 -> 24 mantissa bits suffice).
  - One DVE reduce_min over 16-wide k-chunks -> 16 packed chunk minima
    per point. Level-2 on [*, 16]: global min, equality bitmask dotted
    with 2^j, lowest-set-bit -> chunk index j*; payload of the min ->
    i*; code = 16*j* + i*.
Ties resolve to the smallest k (payload ordering + lowest-set-bit),
matching jnp.argmin's first-index semantics.
"""

import sys

for _p in ("/opt/trn_rl_repo", "/root/.axon_site/_ro/trn_rl_repo"):
    if _p not in sys.path:
        sys.path.insert(0, _p)

import numpy as np

import concourse.bass as bass
import concourse.bacc as bacc
import concourse.mybir as mybir
from concourse import tile
from concourse.bass_utils import run_bass_kernel_spmd

F32 = mybir.dt.float32
BF16 = mybir.dt.bfloat16
I32 = mybir.dt.int32

N_CORES = 8
T_TOTAL = 16
N_SLICES = T_TOTAL // N_CORES          # t-slices per core
POINTS = 128 * 128                     # points per t-slice
N_CHUNKS = 4                           # point-chunks per slice
CHUNK_PTS = POINTS // N_CHUNKS         # 4096
TILES_PER_CHUNK = CHUNK_PTS // 128     # 32
K = 256
D = 64

B_BIG = 3072.0                         # 2^11+2^10: w+B in [2048,4096) -> quantum 2^-12
DELTA = 2.0 ** -16                     # payload step for i = k % 16
PAYSCALE = 2.0 ** 16

AluOp = mybir.AluOpType
Axis = mybir.AxisListType


def _build_nc():
    nc = bacc.Bacc(None, target_bir_lowering=False, debug=False)

    zp_d = nc.declare_dram_parameter(
        "z_planes", [N_SLICES, 2, D + 2, POINTS], BF16, isOutput=False
    )
    mov_d = nc.declare_dram_parameter("mov", [3, D + 2, K], BF16, isOutput=False)
    c23_d = nc.declare_dram_parameter("c23", [1, 2 * K], BF16, isOutput=False)
    pow2_d = nc.declare_dram_parameter("pow2", [128, 16], F32, isOutput=False)
    pay_d = nc.declare_dram_parameter("paycst", [128, 2 * K], F32, isOutput=False)
    codes_d = nc.declare_dram_parameter(
        "codes", [N_SLICES, 128, N_CHUNKS, TILES_PER_CHUNK], I32, isOutput=True
    )

    with tile.TileContext(nc) as tc:
        with (
            tc.tile_pool(name="cst", bufs=1) as cst_pool,
            tc.tile_pool(name="padd", bufs=4) as padd_pool,
            tc.tile_pool(name="chunk", bufs=3) as chunk_pool,
            tc.tile_pool(name="psum", bufs=8, space="PSUM") as psum_pool,
            tc.tile_pool(name="m16", bufs=2) as m16_pool,
            tc.tile_pool(name="l2", bufs=2) as l2_pool,
            tc.tile_pool(name="codes", bufs=2) as codes_pool,
        ):
            m1 = cst_pool.tile([D + 2, K], BF16, tag="m1")
            m2 = cst_pool.tile([D + 2, K], BF16, tag="m2")
            m3 = cst_pool.tile([D + 2, K], BF16, tag="m3")
            # rank-1 moving row lives at partition D (=64) to match the
            # ones-row stationary's base partition (matmul requirement).
            cr = cst_pool.tile([D + 1, 2 * K], BF16, tag="cr")
            c3 = cr[D : D + 1, 0 : 2 * K]
            pow2 = cst_pool.tile([128, 16], F32, tag="pow2")
            paycst = cst_pool.tile([128, 2 * K], F32, tag="paycst")
            nc.sync.dma_start(m1[:], mov_d[0])
            nc.sync.dma_start(m2[:], mov_d[1])
            nc.sync.dma_start(m3[:], mov_d[2])
            nc.sync.dma_start(c3, c23_d[:])
            nc.sync.dma_start(pow2[:], pow2_d[:])
            nc.sync.dma_start(paycst[:], pay_d[:])

            for s in range(N_SLICES):
                codes_sb = codes_pool.tile([128, N_CHUNKS * TILES_PER_CHUNK], I32)
                for c in range(N_CHUNKS):
                    ch_hi = chunk_pool.tile([D + 2, CHUNK_PTS], BF16, tag="ch_hi")
                    ch_mid = chunk_pool.tile([D + 2, CHUNK_PTS], BF16, tag="ch_mid")
                    rng = slice(c * CHUNK_PTS, (c + 1) * CHUNK_PTS)
                    nc.sync.dma_start(ch_hi[:], zp_d[s, 0, :, rng])
                    nc.sync.dma_start(ch_mid[:], zp_d[s, 1, :, rng])
                    # [65, 32, 128]: [:, j, m] = column j + 32*m
                    hi_v = ch_hi[:].rearrange("p (n j) -> p j n", j=TILES_PER_CHUNK)
                    mid_v = ch_mid[:].rearrange("p (n j) -> p j n", j=TILES_PER_CHUNK)
                    ones_ap = hi_v[D : D + 1, 0, :]

                    m16 = m16_pool.tile([128, TILES_PER_CHUNK, 16], F32)
                    for p in range(TILES_PER_CHUNK // 2):
                        ps = psum_pool.tile([128, 512], F32)
                        regs = [ps[:, 0:256], ps[:, 256:512]]
                        js = [2 * p, 2 * p + 1]
                        # Interleave the two tiles' passes so consecutive
                        # matmuls hit different PSUM cells (drain hides
                        # under the other tile's fill). Groups overlap in
                        # the bank -> skip the sim's group lint.
                        for h in range(2):
                            # start=True zeroes the WHOLE bank, so only the
                            # first matmul of the bank may carry it; the
                            # second tile's first write lands on zeroed
                            # cells (has_written=0 -> store).
                            nc.tensor.matmul(
                                regs[h],
                                hi_v[0 : D + 1, js[h], :],
                                m1[0 : D + 1, :],
                                start=(h == 0),
                                stop=False,
                                skip_group_check=True,
                            )
                        for h in range(2):
                            nc.tensor.matmul(
                                regs[h],
                                hi_v[0 : D + 1, js[h], :],
                                m2[0 : D + 1, :],
                                start=False,
                                stop=False,
                                skip_group_check=True,
                            )
                        # 66-row pass: rows 64/65 carry c_k and -B_BIG.
                        # The -B_BIG row accumulates last (ascending row
                        # order), quantizing this pass's content at 2^-12
                        # before the PSUM recenter.
                        route = "VVVGGGGG"[p % 8]
                        for h in range(2):
                            nc.tensor.matmul(
                                regs[h],
                                mid_v[:, js[h], :],
                                m3[:],
                                start=False,
                                stop=(h == 1),
                                skip_group_check=True,
                            )
                        mslc = m16[:, 2 * p : 2 * p + 2, :]
                        if route == "V":
                            # DVE carries the payload add.
                            sc = padd_pool.tile([128, 512], F32, tag="scv")
                            nc.vector.tensor_tensor(
                                sc[:], ps[:], paycst[:], op=AluOp.add
                            )
                        else:
                            # ACT copies PSUM out, GpSimd adds the payload;
                            # DVE only reduces (its PSUM port stays free).
                            sc1 = padd_pool.tile([128, 512], F32, tag="sc1")
                            nc.scalar.activation(
                                sc1[:], ps[:], mybir.ActivationFunctionType.Copy
                            )
                            sc = padd_pool.tile([128, 512], F32, tag="sc2")
                            nc.gpsimd.tensor_tensor(
                                sc[:], sc1[:], paycst[:], op=AluOp.add
                            )
                        nc.vector.tensor_reduce(
                            mslc,
                            sc[:].rearrange("m (t c i) -> m t c i", t=2, c=16, i=16),
                            axis=Axis.X,
                            op=AluOp.min,
                        )

                    # ---- level 2: 32 tiles at once ----
                    vmin = l2_pool.tile([128, TILES_PER_CHUNK], F32)
                    nc.vector.tensor_reduce(vmin[:], m16[:], axis=Axis.X, op=AluOp.min)
                    eq = l2_pool.tile([128, TILES_PER_CHUNK, 16], F32)
                    nc.vector.tensor_tensor(
                        eq[:],
                        m16[:],
                        vmin[:].unsqueeze(2).broadcast_to([128, TILES_PER_CHUNK, 16]),
                        op=AluOp.is_equal,
                    )
                    bs = l2_pool.tile([128, TILES_PER_CHUNK, 16], F32)
                    nc.vector.tensor_tensor(
                        bs[:],
                        eq[:],
                        pow2[:].unsqueeze(1).broadcast_to([128, TILES_PER_CHUNK, 16]),
                        op=AluOp.mult,
                    )
                    b = l2_pool.tile([128, TILES_PER_CHUNK], F32)
                    nc.vector.tensor_reduce(b[:], bs[:], axis=Axis.X, op=AluOp.add)
                    bi = l2_pool.tile([128, TILES_PER_CHUNK], I32)
                    nc.vector.tensor_copy(bi[:], b[:])
                    nbi = l2_pool.tile([128, TILES_PER_CHUNK], I32)
                    nc.vector.tensor_scalar(nbi[:], b[:], -1.0, None, AluOp.mult)
                    low = l2_pool.tile([128, TILES_PER_CHUNK], I32)
                    nc.vector.tensor_tensor(low[:], bi[:], nbi[:], op=AluOp.bitwise_and)
                    lowf = l2_pool.tile([128, TILES_PER_CHUNK], F32)
                    nc.vector.tensor_copy(lowf[:], low[:])
                    # j* = (float_bits(2^j) >> 23) - 127 ; jv = 16*j*
                    jt = l2_pool.tile([128, TILES_PER_CHUNK], I32)
                    nc.vector.tensor_scalar(
                        jt[:], lowf[:].bitcast(I32), 23, None, AluOp.arith_shift_right
                    )
                    jv = l2_pool.tile([128, TILES_PER_CHUNK], I32)
                    nc.vector.tensor_scalar(
                        jv[:], jt[:], 127, 16, AluOp.subtract, AluOp.mult
                    )
                    # i* = (int(vmin * 2^16)) & 15
                    t1 = l2_pool.tile([128, TILES_PER_CHUNK], I32)
                    nc.vector.tensor_scalar(t1[:], vmin[:], PAYSCALE, None, AluOp.mult)
                    t2 = l2_pool.tile([128, TILES_PER_CHUNK], I32)
                    nc.vector.tensor_scalar(t2[:], t1[:], 15, None, AluOp.bitwise_and)
                    nc.vector.tensor_tensor(
                        codes_sb[:, c * TILES_PER_CHUNK : (c + 1) * TILES_PER_CHUNK],
                        jv[:],
                        t2[:],
                        op=AluOp.add,
                    )
                nc.sync.dma_start(
                    codes_d[s],
                    codes_sb[:].rearrange("m (c j) -> m c j", c=N_CHUNKS),
                )
    nc.compile()
    return nc


def _bf16_rn(v: np.ndarray) -> np.ndarray:
    """Round fp32 to bf16 (round-to-nearest-even), returned as fp32 values."""
    u = np.ascontiguousarray(v, dtype=np.float32).view(np.uint32)
    r = (u + np.uint32(0x7FFF) + ((u >> np.uint32(16)) & np.uint32(1))) & np.uint32(
        0xFFFF0000
    )
    return r.view(np.float32)


def _to_bf16(v: np.ndarray) -> np.ndarray:
    import ml_dtypes

    return np.asarray(v, dtype=np.float32).astype(ml_dtypes.bfloat16)


def _make_consts(emb: np.ndarray):
    e2 = (emb.astype(np.float64) ** 2).sum(axis=-1)
    E = (-2.0 * emb.T).astype(np.float32)          # [64, 256]
    eh = _bf16_rn(E)
    em = _bf16_rn(E - eh)
    v64 = (e2 + B_BIG).astype(np.float64)
    av = _bf16_rn(v64.astype(np.float32))
    bv = _bf16_rn((v64 - av.astype(np.float64)).astype(np.float32))
    cv = _bf16_rn((v64 - av.astype(np.float64) - bv).astype(np.float32))

    mov = np.zeros((3, D + 2, K), dtype=np.float32)
    mov[0, 0:D] = eh
    mov[0, D] = av
    mov[1, 0:D] = em
    mov[1, D] = bv
    mov[2, 0:D] = eh
    mov[2, D] = cv
    mov[2, D + 1] = -B_BIG

    c23 = np.zeros((1, 2 * K), dtype=np.float32)
    c23[0, :] = (np.arange(2 * K) % 16).astype(np.float32) * DELTA

    pow2 = np.broadcast_to(
        (2.0 ** np.arange(16)).astype(np.float32), (128, 16)
    ).copy()
    paycst = np.broadcast_to(c23[0:1, :], (128, 2 * K)).copy()
    return _to_bf16(mov), _to_bf16(c23), pow2, paycst


def _pack_z(zr: np.ndarray) -> np.ndarray:
    """zr [S, 64, POINTS] f32 -> [S, 2, 65, POINTS] bf16 (hi/mid planes,
    ones row at partition 64 of both planes)."""
    S = zr.shape[0]
    zh = _bf16_rn(zr)
    zm = _bf16_rn(zr - zh)
    out = np.ones((S, 2, D + 2, POINTS), dtype=np.float32)
    out[:, 0, 0:D] = zh
    out[:, 1, 0:D] = zm
    return _to_bf16(out)


def _run(z: np.ndarray, emb: np.ndarray, **spmd_kwargs):
    z = np.asarray(z, dtype=np.float32)
    emb = np.asarray(emb, dtype=np.float32)
    t, a, b, c = z.shape
    assert (t, a, b, c) == (16, 64, 128, 128) and emb.shape == (256, 64)

    zr = z.reshape(t, a, b * c)
    mov, c23, pow2, paycst = _make_consts(emb)

    nc = _build_nc()
    in_maps = []
    for i in range(N_CORES):
        shard = _pack_z(zr[i * N_SLICES : (i + 1) * N_SLICES])
        in_maps.append(
            {
                "z_planes": shard,
                "mov": mov,
                "c23": c23,
                "pow2": pow2,
                "paycst": paycst,
            }
        )
    res = run_bass_kernel_spmd(nc, in_maps, core_ids=list(range(N_CORES)), **spmd_kwargs)

    out = np.empty((t, b * c), dtype=np.int32)
    for i in range(N_CORES):
        arr = np.asarray(res.results[i]["codes"])  # [N_SLICES, 128, N_CHUNKS, 32]
        # point p = 4096*c + 32*m + j  ->  [s, c, m, j] order is p-major
        out[i * N_SLICES : (i + 1) * N_SLICES] = (
            arr.transpose(0, 2, 1, 3).reshape(N_SLICES, b * c).astype(np.int32)
        )
    return out.reshape(t, b, c), res


def kernel(z: np.ndarray, emb: np.ndarray) -> np.ndarray:
    return _run(z, emb)[0]


# revision 19
# speedup vs baseline: 1.0088x; 1.0034x over previous
"""VQ codebook nearest-neighbor kernel for Trainium2 (8 NeuronCores, SPMD).

Problem: z [16, 64, 128, 128] f32, emb [256, 64] f32 ->
         codes [16, 128, 128] int32 = argmin_k ||x_p - emb_k||_2
         (x = z rearranged 't a b c -> t (b c) a').

Strategy (data-parallel over t, 2 t-slices per core):
  - argmin_k dist = argmin_k (|e_k|^2 - 2 x.e_k)  [x^2 and sqrt are
    monotone per-point -> dropped].
  - Inputs are split host-side into bf16 hi+mid pairs (Dekker-style):
    z = zh + zm + O(2^-17), -2*emb^T = eh + em + O(2^-17). Three bf16
    matmul passes per 128-point tile (zh.eh, zh.em, zm.eh) reproduce the
    fp32 product to ~1e-4 at 1 cycle/column on the PE (vs 4 for fp32).
  - The per-k constant |e_k|^2 + B_BIG is split into three bf16 parts
    a+b+c riding each pass's ones-row. PSUM accumulation in fp32 at
    magnitude ~3072 quantizes w+B_BIG to 2^-12 (B_BIG pins the binade).
  - Two bf16 rank-1 accumulation passes: -B_BIG (recenter, exact) then
    +(k%16)*2^-16 (pack the 4-bit within-chunk index below the quantum;
    exact because |w_q| < 256 -> 24 mantissa bits suffice).
  - One DVE reduce_min over 16-wide k-chunks -> 16 packed chunk minima
    per point. Level-2 on [*, 16]: global min, equality bitmask dotted
    with 2^j, lowest-set-bit -> chunk index j*; payload of the min ->
    i*; code = 16*j* + i*.
Ties resolve to the smallest k (payload ordering + lowest-set-bit),
matching jnp.argmin's first-index semantics.
"""

import sys

for _p in ("/opt/trn_rl_repo", "/root/.axon_site/_ro/trn_rl_repo"):
    if _p not in sys.path:
        sys.path.insert(0, _p)

import numpy as np

import concourse.bass as bass
import concourse.bacc as bacc
import concourse.mybir as mybir
from concourse import tile
from concourse.bass_utils import run_bass_kernel_spmd

F32 = mybir.dt.float32
BF16 = mybir.dt.bfloat16
I32 = mybir.dt.int32

N_CORES = 8
T_TOTAL = 16
N_SLICES = T_TOTAL // N_CORES          # t-slices per core
POINTS = 128 * 128                     # points per t-slice
N_CHUNKS = 4                           # point-chunks per slice
CHUNK_PTS = POINTS // N_CHUNKS         # 4096
TILES_PER_CHUNK = CHUNK_PTS // 128     # 32
K = 256
D = 64

B_BIG = 3072.0                         # 2^11+2^10: w+B in [2048,4096) -> quantum 2^-12
DELTA = 2.0 ** -16                     # payload step for i = k % 16
PAYSCALE = 2.0 ** 16

AluOp = mybir.AluOpType
Axis = mybir.AxisListType


def _build_nc():
    nc = bacc.Bacc(None, target_bir_lowering=False, debug=False)

    zp_d = nc.declare_dram_parameter(
        "z_planes", [N_SLICES, 2, D + 2, POINTS], BF16, isOutput=False
    )
    mov_d = nc.declare_dram_parameter("mov", [3, D + 2, K], BF16, isOutput=False)
    c23_d = nc.declare_dram_parameter("c23", [1, 2 * K], BF16, isOutput=False)
    pow2_d = nc.declare_dram_parameter("pow2", [128, 16], F32, isOutput=False)
    pay_d = nc.declare_dram_parameter("paycst", [128, 2 * K], F32, isOutput=False)
    codes_d = nc.declare_dram_parameter(
        "codes", [N_SLICES, 128, N_CHUNKS, TILES_PER_CHUNK], I32, isOutput=True
    )

    with tile.TileContext(nc) as tc:
        with (
            tc.tile_pool(name="cst", bufs=1) as cst_pool,
            tc.tile_pool(name="padd", bufs=4) as padd_pool,
            tc.tile_pool(name="chunk", bufs=4) as chunk_pool,
            tc.tile_pool(name="psum", bufs=8, space="PSUM") as psum_pool,
            tc.tile_pool(name="m16", bufs=3) as m16_pool,
            tc.tile_pool(name="l2", bufs=2) as l2_pool,
            tc.tile_pool(name="codes", bufs=2) as codes_pool,
        ):
            m1 = cst_pool.tile([D + 2, K], BF16, tag="m1")
            m2 = cst_pool.tile([D + 2, K], BF16, tag="m2")
            m3 = cst_pool.tile([D + 2, K], BF16, tag="m3")
            # rank-1 moving row lives at partition D (=64) to match the
            # ones-row stationary's base partition (matmul requirement).
            cr = cst_pool.tile([D + 1, 2 * K], BF16, tag="cr")
            c3 = cr[D : D + 1, 0 : 2 * K]
            pow2 = cst_pool.tile([128, 16], F32, tag="pow2")
            paycst = cst_pool.tile([128, 2 * K], F32, tag="paycst")
            nc.sync.dma_start(m1[:], mov_d[0])
            nc.sync.dma_start(m2[:], mov_d[1])
            nc.sync.dma_start(m3[:], mov_d[2])
            nc.sync.dma_start(c3, c23_d[:])
            nc.sync.dma_start(pow2[:], pow2_d[:])
            nc.sync.dma_start(paycst[:], pay_d[:])

            for s in range(N_SLICES):
                codes_sb = codes_pool.tile([128, N_CHUNKS * TILES_PER_CHUNK], I32)
                for c in range(N_CHUNKS):
                    ch_hi = chunk_pool.tile([D + 2, CHUNK_PTS], BF16, tag="ch_hi")
                    ch_mid = chunk_pool.tile([D + 2, CHUNK_PTS], BF16, tag="ch_mid")
                    rng = slice(c * CHUNK_PTS, (c + 1) * CHUNK_PTS)
                    nc.sync.dma_start(ch_hi[:], zp_d[s, 0, :, rng])
                    nc.sync.dma_start(ch_mid[:], zp_d[s, 1, :, rng])
                    # [65, 32, 128]: [:, j, m] = column j + 32*m
                    hi_v = ch_hi[:].rearrange("p (n j) -> p j n", j=TILES_PER_CHUNK)
                    mid_v = ch_mid[:].rearrange("p (n j) -> p j n", j=TILES_PER_CHUNK)
                    ones_ap = hi_v[D : D + 1, 0, :]

                    m16 = m16_pool.tile([128, TILES_PER_CHUNK, 16], F32)
                    for p in range(TILES_PER_CHUNK // 2):
                        ps = psum_pool.tile([128, 512], F32)
                        regs = [ps[:, 0:256], ps[:, 256:512]]
                        js = [2 * p, 2 * p + 1]
                        # Interleave the two tiles' passes so consecutive
                        # matmuls hit different PSUM cells (drain hides
                        # under the other tile's fill). Groups overlap in
                        # the bank -> skip the sim's group lint.
                        for h in range(2):
                            # start=True zeroes the WHOLE bank, so only the
                            # first matmul of the bank may carry it; the
                            # second tile's first write lands on zeroed
                            # cells (has_written=0 -> store).
                            nc.tensor.matmul(
                                regs[h],
                                hi_v[0 : D + 1, js[h], :],
                                m1[0 : D + 1, :],
                                start=(h == 0),
                                stop=False,
                                skip_group_check=True,
                            )
                        for h in range(2):
                            nc.tensor.matmul(
                                regs[h],
                                hi_v[0 : D + 1, js[h], :],
                                m2[0 : D + 1, :],
                                start=False,
                                stop=False,
                                skip_group_check=True,
                            )
                        # 66-row pass: rows 64/65 carry c_k and -B_BIG.
                        # The -B_BIG row accumulates last (ascending row
                        # order), quantizing this pass's content at 2^-12
                        # before the PSUM recenter.
                        route = "VVVGGGGG"[p % 8]
                        for h in range(2):
                            nc.tensor.matmul(
                                regs[h],
                                mid_v[:, js[h], :],
                                m3[:],
                                start=False,
                                stop=(h == 1),
                                skip_group_check=True,
                            )
                        mslc = m16[:, 2 * p : 2 * p + 2, :]
                        if route == "V":
                            # DVE carries the payload add.
                            sc = padd_pool.tile([128, 512], F32, tag="scv")
                            nc.vector.tensor_tensor(
                                sc[:], ps[:], paycst[:], op=AluOp.add
                            )
                        else:
                            # ACT copies PSUM out, GpSimd adds the payload;
                            # DVE only reduces (its PSUM port stays free).
                            sc1 = padd_pool.tile([128, 512], F32, tag="sc1")
                            nc.scalar.activation(
                                sc1[:], ps[:], mybir.ActivationFunctionType.Copy
                            )
                            sc = padd_pool.tile([128, 512], F32, tag="sc2")
                            nc.gpsimd.tensor_tensor(
                                sc[:], sc1[:], paycst[:], op=AluOp.add
                            )
                        nc.vector.tensor_reduce(
                            mslc,
                            sc[:].rearrange("m (t c i) -> m t c i", t=2, c=16, i=16),
                            axis=Axis.X,
                            op=AluOp.min,
                        )

                    # ---- level 2: 32 tiles at once ----
                    vmin = l2_pool.tile([128, TILES_PER_CHUNK], F32)
                    nc.vector.tensor_reduce(vmin[:], m16[:], axis=Axis.X, op=AluOp.min)
                    eq = l2_pool.tile([128, TILES_PER_CHUNK, 16], F32)
                    nc.vector.tensor_tensor(
                        eq[:],
                        m16[:],
                        vmin[:].unsqueeze(2).broadcast_to([128, TILES_PER_CHUNK, 16]),
                        op=AluOp.is_equal,
                    )
                    bs = l2_pool.tile([128, TILES_PER_CHUNK, 16], F32)
                    nc.vector.tensor_tensor(
                        bs[:],
                        eq[:],
                        pow2[:].unsqueeze(1).broadcast_to([128, TILES_PER_CHUNK, 16]),
                        op=AluOp.mult,
                    )
                    b = l2_pool.tile([128, TILES_PER_CHUNK], F32)
                    nc.vector.tensor_reduce(b[:], bs[:], axis=Axis.X, op=AluOp.add)
                    bi = l2_pool.tile([128, TILES_PER_CHUNK], I32)
                    nc.vector.tensor_copy(bi[:], b[:])
                    nbi = l2_pool.tile([128, TILES_PER_CHUNK], I32)
                    nc.vector.tensor_scalar(nbi[:], b[:], -1.0, None, AluOp.mult)
                    low = l2_pool.tile([128, TILES_PER_CHUNK], I32)
                    nc.vector.tensor_tensor(low[:], bi[:], nbi[:], op=AluOp.bitwise_and)
                    lowf = l2_pool.tile([128, TILES_PER_CHUNK], F32)
                    nc.vector.tensor_copy(lowf[:], low[:])
                    # j* = (float_bits(2^j) >> 23) - 127 ; jv = 16*j*
                    jt = l2_pool.tile([128, TILES_PER_CHUNK], I32)
                    nc.vector.tensor_scalar(
                        jt[:], lowf[:].bitcast(I32), 23, None, AluOp.arith_shift_right
                    )
                    jv = l2_pool.tile([128, TILES_PER_CHUNK], I32)
                    nc.vector.tensor_scalar(
                        jv[:], jt[:], 127, 16, AluOp.subtract, AluOp.mult
                    )
                    # i* = (int(vmin * 2^16)) & 15
                    t1 = l2_pool.tile([128, TILES_PER_CHUNK], I32)
                    nc.vector.tensor_scalar(t1[:], vmin[:], PAYSCALE, None, AluOp.mult)
                    t2 = l2_pool.tile([128, TILES_PER_CHUNK], I32)
                    nc.vector.tensor_scalar(t2[:], t1[:], 15, None, AluOp.bitwise_and)
                    nc.vector.tensor_tensor(
                        codes_sb[:, c * TILES_PER_CHUNK : (c + 1) * TILES_PER_CHUNK],
                        jv[:],
                        t2[:],
                        op=AluOp.add,
                    )
                nc.sync.dma_start(
                    codes_d[s],
                    codes_sb[:].rearrange("m (c j) -> m c j", c=N_CHUNKS),
                )
    nc.compile()
    return nc


def _bf16_rn(v: np.ndarray) -> np.ndarray:
    """Round fp32 to bf16 (round-to-nearest-even), returned as fp32 values."""
    u = np.ascontiguousarray(v, dtype=np.float32).view(np.uint32)
    r = (u + np.uint32(0x7FFF) + ((u >> np.uint32(16)) & np.uint32(1))) & np.uint32(
        0xFFFF0000
    )
    return r.view(np.float32)


def _to_bf16(v: np.ndarray) -> np.ndarray:
    import ml_dtypes

    return np.asarray(v, dtype=np.float32).astype(ml_dtypes.bfloat16)


def _make_consts(emb: np.ndarray):
    e2 = (emb.astype(np.float64) ** 2).sum(axis=-1)
    E = (-2.0 * emb.T).astype(np.float32)          # [64, 256]
    eh = _bf16_rn(E)
    em = _bf16_rn(E - eh)
    v64 = (e2 + B_BIG).astype(np.float64)
    av = _bf16_rn(v64.astype(np.float32))
    bv = _bf16_rn((v64 - av.astype(np.float64)).astype(np.float32))
    cv = _bf16_rn((v64 - av.astype(np.float64) - bv).astype(np.float32))

    mov = np.zeros((3, D + 2, K), dtype=np.float32)
    mov[0, 0:D] = eh
    mov[0, D] = av
    mov[1, 0:D] = em
    mov[1, D] = bv
    mov[2, 0:D] = eh
    mov[2, D] = cv
    mov[2, D + 1] = -B_BIG

    c23 = np.zeros((1, 2 * K), dtype=np.float32)
    c23[0, :] = (np.arange(2 * K) % 16).astype(np.float32) * DELTA

    pow2 = np.broadcast_to(
        (2.0 ** np.arange(16)).astype(np.float32), (128, 16)
    ).copy()
    paycst = np.broadcast_to(c23[0:1, :], (128, 2 * K)).copy()
    return _to_bf16(mov), _to_bf16(c23), pow2, paycst


def _pack_z(zr: np.ndarray) -> np.ndarray:
    """zr [S, 64, POINTS] f32 -> [S, 2, 65, POINTS] bf16 (hi/mid planes,
    ones row at partition 64 of both planes)."""
    S = zr.shape[0]
    zh = _bf16_rn(zr)
    zm = _bf16_rn(zr - zh)
    out = np.ones((S, 2, D + 2, POINTS), dtype=np.float32)
    out[:, 0, 0:D] = zh
    out[:, 1, 0:D] = zm
    return _to_bf16(out)


def _run(z: np.ndarray, emb: np.ndarray, **spmd_kwargs):
    z = np.asarray(z, dtype=np.float32)
    emb = np.asarray(emb, dtype=np.float32)
    t, a, b, c = z.shape
    assert (t, a, b, c) == (16, 64, 128, 128) and emb.shape == (256, 64)

    zr = z.reshape(t, a, b * c)
    mov, c23, pow2, paycst = _make_consts(emb)

    nc = _build_nc()
    in_maps = []
    for i in range(N_CORES):
        shard = _pack_z(zr[i * N_SLICES : (i + 1) * N_SLICES])
        in_maps.append(
            {
                "z_planes": shard,
                "mov": mov,
                "c23": c23,
                "pow2": pow2,
                "paycst": paycst,
            }
        )
    res = run_bass_kernel_spmd(nc, in_maps, core_ids=list(range(N_CORES)), **spmd_kwargs)

    out = np.empty((t, b * c), dtype=np.int32)
    for i in range(N_CORES):
        arr = np.asarray(res.results[i]["codes"])  # [N_SLICES, 128, N_CHUNKS, 32]
        # point p = 4096*c + 32*m + j  ->  [s, c, m, j] order is p-major
        out[i * N_SLICES : (i + 1) * N_SLICES] = (
            arr.transpose(0, 2, 1, 3).reshape(N_SLICES, b * c).astype(np.int32)
        )
    return out.reshape(t, b, c), res


def kernel(z: np.ndarray, emb: np.ndarray) -> np.ndarray:
    return _run(z, emb)[0]
